# revision 40
# baseline (speedup 1.0000x reference)
"""Multi-head self-attention (B=4, N=1024, D=1024, H=16) on 8 Trainium2 NeuronCores.

Sharding: core c handles batch b = c//2 and head-half hh = c%2 (8 of 16 heads,
512 of 1024 head-dims).  Each core computes Q/K/V projections for its
(batch, head-half), full attention for its 8 heads, and a partial output
projection over its 512 head-dims.  The host sums the partial outputs.

All matmul operands are fp16 (f32 PSUM accumulation).  Layout / algorithm:

  QT[dh, n] = sum_e wq[e, dh] * xT[e, n]          (per head-pair dh-block)
  KT[dh, n] likewise
  V[n, dh]  = sum_e xT[e, n-tile] * wv[e, dh]     v_sb gets a ones column
  eA/eB[k,q]= KT.T @ QT  per head of a pair       two [128, 1024] PSUM tiles
  P[k, q]   = exp(SCALE*e + maskbias - C)         one ACT instr per (pair, kt,
                                                  head); pt fp16
  att[q, d] = sum_k P[k, q-tile] * V'[k, d]       P is the STATIONARY operand:
                                                  out [q, 65] per (head, qtile,
                                                  ktile); col 64 = softmax sum
  at[q, d]  = att * (1/s)                         DVE reciprocal + per-head
                                                  tensor_scalar drains
  atT       = PE transpose of at tiles            for the output projection
  y[n, e]   = sum_dh atT[dh, n-tile] * wo[dh, e]  three partials y01/y2/y3

The att orientation (P stationary, M=128 q-rows) makes P@V' cost 65 columns
per 128x128x65 MAC block instead of 512 — the key PE saving vs a [dh, q]
layout.  eA/eB double-buffering keeps ScalarE 100% busy during the energy/exp
stream; emission interleaves that stream with projections, PV, transposes and
the y-projection so the in-order PE queue never waits on ScalarE.
"""
import os
import sys
import time

for _p in (
    "/opt/trn_rl_repo",
    "/root/.axon_site",
    "/root/.axon_site/_ro/trn_rl_repo",
    "/root/.axon_site/_ro/pypackages",
):
    if os.path.isdir(_p) and _p not in sys.path:
        sys.path.append(_p)

import numpy as np

import concourse.bacc as bacc
import concourse.tile as tile
from concourse import mybir
from concourse.bass_utils import run_bass_kernel_spmd

B, N, D, H = 4, 1024, 1024, 16
DK = D // H          # 64
NCORES = 8
HPC = H // 2         # 8 heads per core
DPC = D // 2         # 512 head-dims per core
NT = N // 128        # 8 token/key tiles
ET = D // 128        # 8 model-dim tiles
SCALE = float(DK) ** -0.5
EXPC = 2.0           # constant shift inside exp; cancels in softmax
MASK_NEG = -30000.0
F32 = mybir.dt.float32
F16 = mybir.dt.float16

_CACHE = {}


def _build():
    nc = bacc.Bacc("TRN2", target_bir_lowering=False, debug=False,
                   num_devices=NCORES)
    xT = nc.dram_tensor("xT", [D, N], F16, kind="ExternalInput")
    wq = nc.dram_tensor("wq", [D, DPC], F16, kind="ExternalInput")
    wk = nc.dram_tensor("wk", [D, DPC], F16, kind="ExternalInput")
    wv = nc.dram_tensor("wv", [D, DPC], F16, kind="ExternalInput")
    wo = nc.dram_tensor("wo", [DPC, D], F16, kind="ExternalInput")
    mb = nc.dram_tensor("mb", [128, NT], F32, kind="ExternalInput")
    idn = nc.dram_tensor("idn", [128, 128], F16, kind="ExternalInput")
    y01 = nc.dram_tensor("y01_part", [N, D], F32, kind="ExternalOutput")
    y23 = nc.dram_tensor("y23_part", [N, D], F32, kind="ExternalOutput")

    with tile.TileContext(nc) as tc:
        with tc.tile_pool(name="sb", bufs=1) as sb, \
             tc.tile_pool(name="work", bufs=2) as wp, \
             tc.tile_pool(name="ps", bufs=2, space="PSUM") as ps:

            # ---------------- persistent SBUF + input loads ----------------
            xT_sb = sb.tile([128, ET, N], F16)
            wq_sb = sb.tile([128, ET, DPC], F16)
            wk_sb = sb.tile([128, ET, DPC], F16)
            wv_sb = sb.tile([128, ET, DPC], F16)
            wo_sb = sb.tile([128, 4, D], F16)
            mb_sb = sb.tile([128, NT], F32)
            ident = sb.tile([128, 128], F16)

            # Few BIG strided DMAs: the queue issue slot (~500ns each)
            # dominates, so critical tiles ship as merged transfers.
            # sync: wq pair0, xT-h0 (2 chunks), wq rest;
            # scalar: wk pair0, xT-h1 (2), wk rest; gpsimd: mb, wv, ident, wo.
            wqr = wq.ap().rearrange("(e p) d -> p e d", p=128)
            wkr = wk.ap().rearrange("(e p) d -> p e d", p=128)
            wvr = wv.ap().rearrange("(e p) d -> p e d", p=128)
            xr = xT.ap().rearrange("(e p) n -> p e n", p=128)
            nc.gpsimd.dma_start(out=mb_sb, in_=mb.ap())
            nc.sync.dma_start(out=wq_sb[:, 0:2, 0:128], in_=wqr[:, 0:2, 0:128])
            nc.scalar.dma_start(out=wk_sb[:, 0:2, 0:128],
                                in_=wkr[:, 0:2, 0:128])
            nc.sync.dma_start(out=xT_sb[:, 0:2, 0:512], in_=xr[:, 0:2, 0:512])
            nc.scalar.dma_start(out=xT_sb[:, 0:2, 512:1024],
                                in_=xr[:, 0:2, 512:1024])
            nc.sync.dma_start(out=wq_sb[:, 2:8, 0:128], in_=wqr[:, 2:8, 0:128])
            nc.scalar.dma_start(out=wk_sb[:, 2:8, 0:128],
                                in_=wkr[:, 2:8, 0:128])
            nc.sync.dma_start(out=xT_sb[:, 2:4, 0:512], in_=xr[:, 2:4, 0:512])
            nc.scalar.dma_start(out=xT_sb[:, 2:4, 512:1024],
                                in_=xr[:, 2:4, 512:1024])
            nc.gpsimd.dma_start(out=xT_sb[:, 4:8, 0:512],
                                in_=xr[:, 4:8, 0:512])
            nc.gpsimd.dma_start(out=xT_sb[:, 4:8, 512:1024],
                                in_=xr[:, 4:8, 512:1024])
            nc.sync.dma_start(out=wq_sb[:, :, 128:512], in_=wqr[:, :, 128:512])
            nc.scalar.dma_start(out=wk_sb[:, :, 128:512],
                                in_=wkr[:, :, 128:512])
            nc.gpsimd.dma_start(out=wv_sb, in_=wvr)
            nc.gpsimd.dma_start(out=ident, in_=idn.ap())
            nc.gpsimd.dma_start(
                out=wo_sb, in_=wo.ap().rearrange("(t p) d -> p t d", p=128))

            qt_sb = sb.tile([128, 4, N], F16)
            kt_sb = sb.tile([128, 4, N], F16)
            v_sb = sb.tile([128, NT, HPC, DK + 1], F16)
            at_sb = sb.tile([128, NT, 4, 128], F16)
            atT_sb = sb.tile([128, 4, N], F16)

            pt = {}
            pv_t = {}

            # ---------------- Q/K projection for head pair p ----------------
            def qkproj(p, w_sb, dst, half):
                qs = slice(half * 512, (half + 1) * 512)
                t = ps.tile([128, 512], F32, tag="py",
                            name=f"qk{p}_{half}_{dst is kt_sb}")
                for et in range(ET):
                    nc.tensor.matmul(t, w_sb[:, et, p * 128:(p + 1) * 128],
                                     xT_sb[:, et, qs],
                                     start=(et == 0), stop=(et == ET - 1))
                nc.vector.tensor_copy(out=dst[:, p, qs], in_=t)

            # ---------------- V projection for token tile t -----------------
            def vproj(t_):
                pvt = ps.tile([128, 512], F32, tag="py", name=f"v{t_}")
                for et in range(ET):
                    nc.tensor.matmul(pvt,
                                     xT_sb[:, et, t_ * 128:(t_ + 1) * 128],
                                     wv_sb[:, et, :],
                                     start=(et == 0), stop=(et == ET - 1))
                nc.vector.tensor_copy(
                    out=v_sb[:, t_, :, 0:DK],
                    in_=pvt.rearrange("p (h d) -> p h d", h=HPC))
                nc.vector.memset(v_sb[:, t_, :, DK:DK + 1], 1.0)

            # ------------- energies + exp for (pair, key tile) --------------
            # eA/eB double-buffered [128, 1024] tiles: exp(kt) of head A
            # overlaps the energy matmuls of head B / the next kt, keeping
            # ScalarE 100% busy during the stream (a single [128, 2048] tile
            # serialized E-after-exp-after-E through the slot WAR).
            def E(p, kt):
                ks = slice(kt * 128, (kt + 1) * 128)
                for h01 in range(2):
                    e_t = ps.tile([128, 1024], F32, tag="e", bufs=2,
                                  name=f"e{p}_{kt}_{h01}")
                    po = slice(h01 * 64, (h01 + 1) * 64)
                    for half in range(2):
                        qs = slice(half * 512, (half + 1) * 512)
                        nc.tensor.matmul(
                            e_t[:, half * 512:(half + 1) * 512],
                            kt_sb[po, p, ks], qt_sb[po, p, qs],
                            start=True, stop=True)
                    nc.scalar.activation(
                        pt[p][:, kt, h01 * 1024:(h01 + 1) * 1024], e_t,
                        mybir.ActivationFunctionType.Exp,
                        bias=mb_sb[:, kt:kt + 1], scale=SCALE)

            def pt_alloc(p):
                pt[p] = wp.tile([128, NT, 2048], F16, tag="pt", bufs=2,
                                name=f"pt{p}")

            # ---------- P @ V' for (pair, q tile): out [q, 2, 65] -----------
            # col 64 of each head's 65-block accumulates the softmax sum.
            # pvbank: three PV accumulation groups share one PSUM bank.
            # Groups are emitted contiguously, so a later group's start only
            # zero-region-poisons groups that have fully accumulated (their
            # values stay valid for the fin() drain).
            pvbank = ps.tile([128, 3, 2, DK + 1], F32, tag="pv", bufs=1,
                             name="pvbank")
            tbank = ps.tile([128, 2, 128], F16, tag="tb", bufs=1,
                            name="tbank")

            def PV(p, qt):
                t = pvbank[:, qt % 3]
                pv_t[(p, qt)] = t
                for h01 in range(2):
                    for kt in range(NT):
                        nc.tensor.matmul(
                            t[:, h01, :],
                            pt[p][:, kt, h01 * 1024 + qt * 128:
                                  h01 * 1024 + (qt + 1) * 128],
                            v_sb[:, kt, 2 * p + h01, :],
                            start=(kt == 0), stop=(kt == NT - 1))

            # -------- softmax normalization: at = att * (1/s) ---------------
            def fin(p, qt):
                t = pv_t.pop((p, qt))
                rs = wp.tile([128, 2, 1], F32, tag="rs", bufs=4,
                             name=f"rs{p}_{qt}")
                nc.vector.reciprocal(rs, t[:, :, DK:DK + 1])
                for h01 in range(2):
                    nc.vector.tensor_scalar_mul(
                        at_sb[:, qt, p, h01 * 64:(h01 + 1) * 64],
                        t[:, h01, 0:DK], rs[:, h01, :])

            # ------------- transpose at [q, dh] -> atT [dh, q] --------------
            def T(p, qt):
                tp = tbank[:, qt % 2]
                nc.tensor.transpose(tp, at_sb[:, qt, p, :], ident)
                nc.vector.tensor_copy(
                    out=atT_sb[:, p, qt * 128:(qt + 1) * 128], in_=tp)

            # ---------------- output projection partials --------------------
            def ygrp(nt, eh, dts, ydram, eng):
                yp = ps.tile([128, 512], F32, tag="py",
                             name=f"y{dts[0]}_{nt}_{eh}")
                ns = slice(nt * 128, (nt + 1) * 128)
                es = slice(eh * 512, (eh + 1) * 512)
                for i, dt in enumerate(dts):
                    nc.tensor.matmul(yp, atT_sb[:, dt, ns], wo_sb[:, dt, es],
                                     start=(i == 0), stop=(i == len(dts) - 1))
                ys = wp.tile([128, 512], F32, tag="ysb", bufs=6,
                             name=f"ys{dts[0]}_{nt}_{eh}")
                if eng in ("act", "last"):
                    nc.scalar.copy(ys, yp)
                else:
                    nc.vector.tensor_copy(out=ys, in_=yp)
                if eng == "last":
                    # final unit: split across both queues so the kernel-end
                    # barrier waits on two half-size transfers
                    nc.sync.dma_start(out=ydram.ap()[ns, eh * 512:eh * 512 + 256],
                                      in_=ys[:, 0:256])
                    nc.gpsimd.dma_start(
                        out=ydram.ap()[ns, eh * 512 + 256:(eh + 1) * 512],
                        in_=ys[:, 256:512])
                else:
                    q = nc.sync if eh == 0 else nc.gpsimd
                    q.dma_start(out=ydram.ap()[ns, es], in_=ys)

            def y01u(nt, eh, eng="vector"):
                ygrp(nt, eh, (0, 1), y01, eng)

            def y23u(nt, eh, eng="act"):
                # tail: the exp stream is finished, ScalarE mostly free
                ygrp(nt, eh, (2, 3), y23, eng)

            # ------------- emission order (software pipeline) ---------------
            # Each E(p, kt) slot carries ~2us of independent PE work so the
            # in-order PE queue never waits on the ScalarE exp stream.
            qkproj(0, wq_sb, qt_sb, 0); qkproj(0, wq_sb, qt_sb, 1)
            qkproj(0, wk_sb, kt_sb, 0); qkproj(0, wk_sb, kt_sb, 1)
            pt_alloc(0)
            E(0, 0); vproj(0); vproj(1)
            E(0, 1); vproj(2); vproj(3)
            E(0, 2); vproj(4); vproj(5)
            E(0, 3); vproj(6); vproj(7)
            E(0, 4); qkproj(1, wq_sb, qt_sb, 0)
            E(0, 5); qkproj(1, wq_sb, qt_sb, 1)
            E(0, 6); qkproj(1, wk_sb, kt_sb, 0)
            E(0, 7); qkproj(1, wk_sb, kt_sb, 1)

            pt_alloc(1)
            E(1, 0); PV(0, 0)
            E(1, 1); fin(0, 0); PV(0, 1); qkproj(2, wq_sb, qt_sb, 0)
            E(1, 2); fin(0, 1); PV(0, 2); T(0, 0); qkproj(2, wq_sb, qt_sb, 1)
            E(1, 3); fin(0, 2); PV(0, 3); T(0, 1); qkproj(2, wk_sb, kt_sb, 0)
            E(1, 4); fin(0, 3); PV(0, 4); T(0, 2); qkproj(2, wk_sb, kt_sb, 1)
            E(1, 5); fin(0, 4); PV(0, 5); T(0, 3); qkproj(3, wq_sb, qt_sb, 0)
            E(1, 6); fin(0, 5); PV(0, 6); T(0, 4); qkproj(3, wq_sb, qt_sb, 1)
            E(1, 7); fin(0, 6); PV(0, 7); T(0, 5); qkproj(3, wk_sb, kt_sb, 0)

            pt_alloc(2)
            E(2, 0); fin(0, 7); T(0, 6); PV(1, 0); qkproj(3, wk_sb, kt_sb, 1)
            E(2, 1); fin(1, 0); T(0, 7); PV(1, 1); T(1, 0)
            E(2, 2); fin(1, 1); PV(1, 2); T(1, 1); y01u(0, 0); y01u(0, 1)
            E(2, 3); fin(1, 2); PV(1, 3); T(1, 2); y01u(1, 0)
            E(2, 4); fin(1, 3); PV(1, 4); T(1, 3); y01u(1, 1); y01u(2, 0)
            E(2, 5); fin(1, 4); PV(1, 5); T(1, 4); y01u(2, 1)
            E(2, 6); fin(1, 5); PV(1, 6); T(1, 5); y01u(3, 0); y01u(3, 1)
            E(2, 7); fin(1, 6); PV(1, 7); T(1, 6); y01u(4, 0)

            pt_alloc(3)
            E(3, 0); fin(1, 7); T(1, 7); PV(2, 0); y01u(4, 1)
            E(3, 1); fin(2, 0); PV(2, 1); T(2, 0); y01u(5, 0)
            E(3, 2); fin(2, 1); PV(2, 2); T(2, 1); y01u(5, 1)
            E(3, 3); fin(2, 2); PV(2, 3); T(2, 2); y01u(6, 0)
            E(3, 4); fin(2, 3); PV(2, 4); T(2, 3); y01u(6, 1)
            E(3, 5); fin(2, 4); PV(2, 5); T(2, 4); y01u(7, 0)
            E(3, 6); fin(2, 5); PV(2, 6); T(2, 5)
            E(3, 7); fin(2, 6); PV(2, 7); T(2, 6); y01u(7, 1)

            # tail: T lags its fin; inline y23 units drain on ScalarE (free
            # after the exp stream); the final units drain on DVE after the
            # fin chain has been fully emitted
            PV(3, 0); fin(2, 7); T(2, 7)
            PV(3, 1); fin(3, 0); T(3, 0)
            PV(3, 2); fin(3, 1); T(3, 1); y23u(0, 0); y23u(0, 1)
            PV(3, 3); fin(3, 2); T(3, 2); y23u(1, 0); y23u(1, 1)
            PV(3, 4); fin(3, 3); T(3, 3); y23u(2, 0); y23u(2, 1)
            PV(3, 5); fin(3, 4); T(3, 4); y23u(3, 0); y23u(3, 1)
            PV(3, 6); fin(3, 5); T(3, 5); y23u(4, 0); y23u(4, 1)
            PV(3, 7); fin(3, 6); T(3, 6); y23u(5, 0); y23u(5, 1)
            fin(3, 7); T(3, 7)
            y23u(6, 0, "vector"); y23u(6, 1); y23u(7, 0, "vector"); y23u(7, 1, "last")

    nc.compile()
    return nc


def _get_nc():
    if "nc" not in _CACHE:
        _CACHE["nc"] = _build()
    return _CACHE["nc"]


def _f16(a):
    return np.ascontiguousarray(a).astype(np.float16)


def _numpy_fallback(x, mask, Wq, bq, Wk, bk, Wv, bv, Wo, bo):
    # correctness fallback for nonzero q/k/v biases (not hit by the
    # benchmark inputs, which use zero biases)
    out = np.empty((B, N, D), dtype=np.float32)
    scale = np.float32(DK ** -0.5)
    for b in range(B):
        q = (x[b] @ Wq.T + bq).reshape(N, H, DK).transpose(1, 0, 2)
        k = (x[b] @ Wk.T + bk).reshape(N, H, DK).transpose(1, 0, 2)
        v = (x[b] @ Wv.T + bv).reshape(N, H, DK).transpose(1, 0, 2)
        e = np.einsum("hqd,hkd->hqk", q, k) * scale
        e = np.where(mask[b][None, None, :], np.float32(-1e30), e)
        e -= e.max(axis=2, keepdims=True)
        p = np.exp(e)
        p /= p.sum(axis=2, keepdims=True)
        att = np.einsum("hqk,hkd->hqd", p, v)
        out[b] = att.transpose(1, 0, 2).reshape(N, D) @ Wo.T + bo
    return out


def kernel(x, mask, Wq, bq, Wk, bk, Wv, bv, Wo, bo):
    x = np.asarray(x, dtype=np.float32)
    mask = np.asarray(mask)
    Wq = np.asarray(Wq, dtype=np.float32)
    Wk = np.asarray(Wk, dtype=np.float32)
    Wv = np.asarray(Wv, dtype=np.float32)
    Wo = np.asarray(Wo, dtype=np.float32)
    bq = np.asarray(bq, dtype=np.float32)
    bk = np.asarray(bk, dtype=np.float32)
    bv = np.asarray(bv, dtype=np.float32)
    bo = np.asarray(bo, dtype=np.float32)

    if np.any(bq) or np.any(bk) or np.any(bv):
        return _numpy_fallback(x, mask, Wq, bq, Wk, bk, Wv, bv, Wo, bo)

    nc = _get_nc()
    ident = np.eye(128, dtype=np.float16)

    in_maps = []
    for c in range(NCORES):
        b = c // 2
        hh = c % 2
        dsl = slice(hh * DPC, (hh + 1) * DPC)
        mbias = (np.where(mask[b], MASK_NEG, 0.0) - EXPC).astype(np.float32)
        in_maps.append({
            "xT": _f16(x[b].T),
            "wq": _f16(Wq[dsl, :].T),
            "wk": _f16(Wk[dsl, :].T),
            "wv": _f16(Wv[dsl, :].T),
            "wo": _f16(Wo[:, dsl].T),
            "mb": np.ascontiguousarray(mbias.reshape(NT, 128).T),
            "idn": ident,
        })

    res = None
    for attempt in range(3):
        try:
            res = run_bass_kernel_spmd(nc, in_maps,
                                       core_ids=list(range(NCORES)))
            break
        except Exception:
            # transient NRT/axon failures recover on retry
            if attempt == 2:
                raise
            time.sleep(2.0)

    out = np.empty((B, N, D), dtype=np.float32)
    for b in range(B):
        r0 = res.results[2 * b]
        r1 = res.results[2 * b + 1]
        out[b] = ((r0["y01_part"] + r0["y23_part"])
                  + (r1["y01_part"] + r1["y23_part"]) + bo)
    return out
